# revision 30
# baseline (speedup 1.0000x reference)
"""Trainium2 Bass kernel for nn_Mhsa_47802986004933.

Model (per batch b of 2):
  BN(train-stats)+ReLU -> 1x1 conv qkv (raw .view reinterpret) ->
  4-head attention on heads 0-3  +  conv-mixing (3x1 / 1x3) on heads 4-7 ->
  concat -> kernel-2 avg pool.

Sharding: 8 cores = (batch b in {0,1}) x (h in {0..3}).
  Core c = 4b + h:
    - full 4096x4096 attention for head h of batch b  -> out[b, :, 32h:32h+32]
    - conv y-quarter [16h, 16h+16)                    -> out[b, n%16 in [4h,4h+4), 128:256]
  Communication-free SPMD: BN stats recomputed on every core from the full x.

Key structural identity: with O = W @ xn [1536, 4096] per batch and
U = O.reshape(12288, 512) (u = 8o+g), token n has q = U[3n], k = U[3n+1],
v = U[3n+2].  Attention head h uses U columns [64h, 64h+64); the conv branch
uses columns [256, 512) with image layout q2[i, y, x] =
U[3*(64*(i%64)+y), 256 + 64*(i//64) + x].

Perf structure (vs the 150 us baseline):
  - Q/K/V head projections are drained into ut as fp8e4m3 scaled by 0.25;
    QK^T runs as a DoubleRow fp8 matmul with a stride-0 "pair" dim (result
    is doubled, so psum = q.k/8 exactly) -> 2x PE throughput on the
    dominant matmul.  V pair-sums also read fp8 (plain fp8 matmul).
  - BN stats are split DVE (bn_stats) + ACT (Copy/Square accumulate) and
    hand-merged, roughly halving the serial stats head.
  - The trailing kernel-2 avg-pool is pushed through the conv branch as a
    spatial pre-pooling (even/odd phase) done on the Pool engine, halving
    the conv matmul and drain cost.
  - PE warm-up matmuls during the head keep the tensor engine's p-state
    ramped so proj/attention matmuls run at full clock.
  - Softmax exp is split across ACT and DVE (Schraudolph int16->bf16 trick,
    bias cancels in the normalizer); denominator rides the AV matmul as a
    33rd ones-column with V pre-pair-summed.
"""
import os
import sys
import numpy as np
import ml_dtypes

sys.path.insert(0, "/opt/trn_rl_repo")

import concourse.bass as bass
import concourse.bacc as bacc
import concourse.mybir as mybir
import concourse.tile as tile
from concourse import bass_utils

B, N, DIM, S = 2, 4096, 256, 64
H, DH, INNER = 8, 64, 512
EPS = 1e-5
FP = mybir.dt.float32
FR = mybir.dt.float32r
BF = mybir.dt.bfloat16
F8 = mybir.dt.float8e4
I32 = mybir.dt.int32
I16 = mybir.dt.int16
AF = mybir.ActivationFunctionType
OP = mybir.AluOpType
PM = mybir.MatmulPerfMode

NG = 16            # score groups per 512-query chunk (2 key-blocks each)
LAG = 3            # AV trails QK by this many groups
# exp engine per group: A=ACT, D=DVE (Pool cannot read PSUM on real TRN2).
# 9A:7D matches the engines' 996/1192 ns per-call rates; runs capped at 2
# and blocks start with D so no 3-run forms across block boundaries.
EXPENG = "DADADADADADAADAA"
assert len(EXPENG) == NG
# Schraudolph fast-exp of scores already scaled to s/8 in PSUM:
# int16 bits = x*EXPA + EXPB, read as bf16
EXPA = (1 << 7) * 1.4426950408889634
EXPB = float(127 << 7) - 366393.0 / 65536.0 + 0.5
ALPHA = 0.25       # fp8 drain scale for q/k/v; DoubleRow doubling gives
                   # psum = 2*(0.25q).(0.25k) = q.k/8
NWARM = 64         # PE p-state warm-up matmuls during the head phase


def _r(ap):
    return ap.bitcast(FR)


def build_device_program():
    nc = bacc.Bacc(
        "TRN2", target_bir_lowering=False, debug=False, enable_asserts=True,
        num_devices=8,
    )
    xts = nc.dram_tensor("xts", [256, 8192], F8, kind="ExternalInput").ap()
    xc_d = nc.dram_tensor("xc", [256, 2560], BF, kind="ExternalInput").ap()
    wq_d = nc.dram_tensor("wq", [256, 1536], BF, kind="ExternalInput").ap()
    wcg_d = nc.dram_tensor("wcg", [256, 3072], BF, kind="ExternalInput").ap()
    wch_d = nc.dram_tensor("wch", [256, 128], BF, kind="ExternalInput").ap()
    w1s_d = nc.dram_tensor("w1s", [256, 768], FP, kind="ExternalInput").ap()
    w2s_d = nc.dram_tensor("w2s", [256, 768], FP, kind="ExternalInput").ap()
    gb_d = nc.dram_tensor("gb", [256, 2], FP, kind="ExternalInput").ap()
    pairm_d = nc.dram_tensor("pairm", [64, 32], F8, kind="ExternalInput").ap()
    out_a = nc.dram_tensor("out_a", [4096, 32], FP, kind="ExternalOutput").ap()
    out_c = nc.dram_tensor("out_c", [1024, 128], FP, kind="ExternalOutput").ap()

    with tile.TileContext(nc) as tc:
        _emit(tc, nc, xts, xc_d, wq_d, wcg_d, wch_d, w1s_d, w2s_d, gb_d,
              pairm_d, out_a, out_c)
    nc.compile()
    return nc


def _emit(tc, nc, xts, xc_d, wq_d, wcg_d, wch_d, w1s_d, w2s_d, gb_d,
          pairm_d, out_a, out_c):
    from contextlib import ExitStack
    ctx = ExitStack()
    with ctx:
        cp = ctx.enter_context(tc.tile_pool(name="const", bufs=1))
        sctx = ExitStack()
        sp = sctx.enter_context(tc.tile_pool(name="scratch", bufs=1))
        xctx = ExitStack()
        xp = xctx.enter_context(tc.tile_pool(name="xload", bufs=1))
        pctx = ExitStack()
        pm = pctx.enter_context(tc.tile_pool(name="ps_m", bufs=6, space="PSUM"))
        wm = pctx.enter_context(tc.tile_pool(name="ps_w", bufs=2, space="PSUM"))

        dma = nc.sync.dma_start
        vec = nc.vector
        act = nc.scalar
        gp = nc.gpsimd

        # ---------------- persistent SBUF ----------------
        ut = cp.tile([64, 12288], F8, tag="ut", name="ut")
        vsb = cp.tile([128, 1056], BF, tag="vsb", name="vsb")
        psbs = [cp.tile([128, 1024], BF, tag=f"psb{k}", name=f"psb{k}")
                for k in range(LAG + 1)]
        resbs = [cp.tile([128, 128], FP, tag=f"resb{k}", name=f"resb{k}")
                 for k in range(2)]
        rec4s = [cp.tile([128, 4], FP, tag=f"rec4{k}", name=f"rec4{k}")
                 for k in range(2)]
        onec = cp.tile([128, 1], FP, tag="onec", name="onec")
        vec.memset(onec, 1.0)
        zconst = cp.tile([128, 1], FP, tag="zconst", name="zconst")
        vec.memset(zconst, 0.0)
        nc.const_aps.aps[(FP, 0.0)] = zconst

        # PE warm-up constants (Pool memsets so no engine on the critical
        # path is touched); keeps the PE p-state ramped during the head.
        warmL = cp.tile([64, 64], BF, tag="warmL", name="warmL")
        warmR = cp.tile([64, 512], BF, tag="warmR", name="warmR")
        gp.memset(warmL, 0.125)
        gp.memset(warmR, 0.125)
        for i in range(NWARM):
            pw = wm.tile([64, 512], FP, tag="w", name=f"warm{i}")
            nc.tensor.matmul(pw, warmL, warmR, start=True, stop=True,
                             skip_group_check=True)

        # conv weights live in cp: used mid/late, after scratch closes
        w1s = [cp.tile([128, 768], FR, tag="w1s0", name="w1s0"),
               cp.tile([128, 768], FR, tag="w1s1", name="w1s1")]
        w2s = [cp.tile([128, 768], FR, tag="w2s0", name="w2s0"),
               cp.tile([128, 768], FR, tag="w2s1", name="w2s1")]
        # scratch-lifetime tiles (front phase)
        wq = [sp.tile([128, 1536], BF, tag="wq0", name="wq0"),
              sp.tile([128, 1536], BF, tag="wq1", name="wq1")]
        gb = [sp.tile([128, 2], FP, tag="gb0", name="gb0"),
              sp.tile([128, 2], FP, tag="gb1", name="gb1")]
        pairm = sp.tile([64, 32], F8, tag="pairm", name="pairm")
        wch = [sp.tile([128, 128], BF, tag="wch0", name="wch0"),
               sp.tile([128, 128], BF, tag="wch1", name="wch1")]
        xcb = [sp.tile([128, 2560], BF, tag="xcb0", name="xcb0"),
               sp.tile([128, 2560], BF, tag="xcb1", name="xcb1")]
        xn = [sp.tile([128, 2560], BF, tag="xn0", name="xn0"),
              sp.tile([128, 2560], BF, tag="xn1", name="xn1")]
        # permute-DMA sources live in the persistent pool so that closing
        # the scratch pool does not barrier on those (long) DMAs
        sq2 = cp.tile([128, 2048], FR, tag="sq2", name="sq2")
        sk2 = cp.tile([128, 2048], FR, tag="sk2", name="sk2")
        sv2 = cp.tile([128, 2048], FR, tag="sv2", name="sv2")
        qhalo = [cp.tile([64, 256], FR, tag="qhalo0", name="qhalo0"),
                 cp.tile([64, 256], FR, tag="qhalo1", name="qhalo1")]
        q2q = [cp.tile([128, 1152], FR, tag="q2q0", name="q2q0"),
               cp.tile([128, 1152], FR, tag="q2q1", name="q2q1")]
        k2q = [cp.tile([128, 1024], FR, tag="k2q0", name="k2q0"),
               cp.tile([128, 1024], FR, tag="k2q1", name="k2q1")]
        v2q = [cp.tile([128, 1024], FR, tag="v2q0", name="v2q0"),
               cp.tile([128, 1024], FR, tag="v2q1", name="v2q1")]
        # spatially pre-pooled conv inputs (Pool engine, overlap attention)
        q2p = [cp.tile([128, 576], FR, tag="q2p0", name="q2p0"),
               cp.tile([128, 576], FR, tag="q2p1", name="q2p1")]
        k2E = [cp.tile([128, 512], FR, tag="k2E0", name="k2E0"),
               cp.tile([128, 512], FR, tag="k2E1", name="k2E1")]
        k2O = [cp.tile([128, 528], FR, tag="k2O0", name="k2O0"),
               cp.tile([128, 528], FR, tag="k2O1", name="k2O1")]
        v2p = [cp.tile([128, 512], FP, tag="v2p0", name="v2p0"),
               cp.tile([128, 512], FP, tag="v2p1", name="v2p1")]
        pavb = [cp.tile([128, 512], FP, tag="pavb0", name="pavb0"),
                cp.tile([128, 512], FP, tag="pavb1", name="pavb1")]

        # ---------------- x load + BN stats (DVE + ACT split) -----------
        # per hf: tiles j0/j1/j3 -> DVE bn_stats (12x512 chunks), j2 -> ACT
        # (Copy+Square accumulate over the whole 2048-col granule); the
        # real-HW BIR verifier rejects accumulate ops on Pool, so Pool sits
        # this out.  Hand-merged afterwards.
        xk = {}
        for hf in range(2):
            for j in range(4):
                xkt = xp.tile([128, 2048], F8, tag="xk", bufs=8,
                              name=f"xk{hf}{j}")
                if hf == 0 and j == 0:
                    for sub in range(4):
                        dma(out=xkt[:, 512 * sub:512 * sub + 512],
                            in_=xts[0:128, 512 * sub:512 * sub + 512])
                else:
                    dma(out=xkt, in_=xts[128 * hf:128 * hf + 128,
                                         2048 * j:2048 * j + 2048])
                xk[(hf, j)] = xkt
        for h2 in range(2):
            dma(out=gb[h2], in_=gb_d[128 * h2:128 * h2 + 128, :])
        epst = cp.tile([128, 1], FP, tag="epst", name="epst")
        vec.memset(epst, EPS)
        trash = xp.tile([128, 2048], FP, tag="trash", name="trash")
        # pin the ACT function table: Sqrt selects sqrt_and_others, which
        # also contains Copy/Square/Relu -- avoids a mid-head 1.3us reload
        act.activation(trash[:, 0:1], epst, AF.Sqrt)
        aff = []
        for hf in range(2):
            bnst = xp.tile([128, 72], FP, tag=f"bnst{hf}", name=f"bnst{hf}")
            sa = xp.tile([128, 2], FP, tag=f"sa{hf}", name=f"sa{hf}")
            for kk in range(11):
                j, sub = (kk // 4, kk % 4) if kk < 8 else (3, kk % 4 + 1)
                src = xk[(hf, j)][:, 512 * sub:512 * sub + 512]
                vec.bn_stats(out=bnst[:, 6 * kk:6 * kk + 6], in_=src)
            sb2 = xp.tile([128, 2], FP, tag=f"sb2{hf}", name=f"sb2{hf}")
            act.activation(trash, xk[(hf, 2)], AF.Copy,
                           accum_out=sa[:, 0:1])
            act.activation(trash, xk[(hf, 2)], AF.Square,
                           accum_out=sa[:, 1:2])
            act.activation(trash[:, 0:512], xk[(hf, 3)][:, 0:512], AF.Copy,
                           accum_out=sb2[:, 0:1])
            act.activation(trash[:, 0:512], xk[(hf, 3)][:, 0:512], AF.Square,
                           accum_out=sb2[:, 1:2])
            vec.tensor_tensor(sa, sa, sb2, OP.add)
            # merge: DVE part (mean, var over 5632) + ACT sums
            mv = xp.tile([128, 2], FP, tag=f"mv{hf}", name=f"mv{hf}")
            vec.bn_aggr(out=mv,
                        in_=bnst[:, 0:66].rearrange("p (k s) -> p k s", s=6))
            nd = 5632.0
            tm = xp.tile([128, 4], FP, tag=f"tm{hf}", name=f"tm{hf}")
            # tm0 = total_sum = mean_d*nd + sum_a
            vec.scalar_tensor_tensor(out=tm[:, 0:1], in0=mv[:, 0:1],
                                     scalar=nd, in1=sa[:, 0:1],
                                     op0=OP.mult, op1=OP.add)
            # tm1 = mean_d^2 + var_d
            vec.tensor_tensor(tm[:, 1:2], mv[:, 0:1], mv[:, 0:1], OP.mult)
            vec.tensor_tensor(tm[:, 1:2], tm[:, 1:2], mv[:, 1:2], OP.add)
            # tm2 = total_sq = tm1*nd + sq_a
            vec.scalar_tensor_tensor(out=tm[:, 2:3], in0=tm[:, 1:2],
                                     scalar=nd, in1=sa[:, 1:2],
                                     op0=OP.mult, op1=OP.add)
            # mean = tm0/8192 ; ex2 = tm2/8192 ; var = ex2 - mean^2
            mean = xp.tile([128, 1], FP, tag=f"mean{hf}", name=f"mean{hf}")
            vec.tensor_scalar(mean, tm[:, 0:1], 1.0 / 8192.0, None, OP.mult)
            var = xp.tile([128, 1], FP, tag=f"var{hf}", name=f"var{hf}")
            vec.tensor_scalar(var, tm[:, 2:3], 1.0 / 8192.0, None, OP.mult)
            vec.tensor_tensor(tm[:, 3:4], mean, mean, OP.mult)
            vec.tensor_tensor(var, var, tm[:, 3:4], OP.subtract)
            # affine: a = gamma*rsqrt(var+eps), b = beta - mean*a
            sqv = xp.tile([128, 1], FP, tag=f"sqv{hf}", name=f"sqv{hf}")
            act.activation(sqv, var, AF.Sqrt, bias=epst)
            rsv = xp.tile([128, 1], FP, tag=f"rsv{hf}", name=f"rsv{hf}")
            vec.reciprocal(rsv, sqv)
            a_ = xp.tile([128, 1], FP, tag=f"a{hf}", name=f"a{hf}")
            vec.tensor_tensor(a_, rsv, gb[hf][:, 0:1], OP.mult)
            tmp = xp.tile([128, 1], FP, tag=f"tmp{hf}", name=f"tmp{hf}")
            vec.tensor_tensor(tmp, mean, a_, OP.mult)
            bb = xp.tile([128, 1], FP, tag=f"bb{hf}", name=f"bb{hf}")
            vec.tensor_tensor(bb, gb[hf][:, 1:2], tmp, OP.subtract)
            aff.append((a_, bb))

        # ---------------- weight loads (wq first: it gates proj) --------
        for hf in range(2):
            dma(out=wq[hf], in_=wq_d[128 * hf:128 * hf + 128, :])
        for hf in range(2):
            dma(out=xcb[hf], in_=xc_d[128 * hf:128 * hf + 128, :])
        dma(out=pairm, in_=pairm_d)
        for hf in range(2):
            dma(out=wch[hf], in_=wch_d[128 * hf:128 * hf + 128, :])

        # ---------------- xn = relu(a*x+b) ----------------
        for hf in range(2):
            a_, bb = aff[hf]
            act.activation(xn[hf], xcb[hf], AF.Relu, bias=bb, scale=a_)
        xctx.close()

        # ---------------- head projection -> ut (fp8, *0.25) ------------
        # paired groups: ps[0:64]=group 2gp, ps[64:128]=group 2gp+1
        # (PSUM readers can only be DVE/ACT; Pool is PSUM-banned.)
        cyc = [vec, act]
        ci = 0

        def drain(dst, src, scale=None):
            nonlocal ci
            eng = cyc[ci % 2]
            ci += 1
            if eng is act:
                if scale is None:
                    act.activation(dst, src, AF.Copy)
                else:
                    act.activation(dst, src, AF.Copy, scale=scale)
            else:
                if scale is None:
                    vec.tensor_copy(dst, src)
                else:
                    vec.tensor_scalar(dst, src, scale, None, OP.mult)

        for gpi in range(4):
            for oc in range(3):
                ps = pm.tile([128, 512], FP, tag="m", name=f"pr{gpi}_{oc}")
                nc.tensor.matmul(ps, xn[0][:, 128 * gpi:128 * gpi + 128],
                                 wq[0][:, 512 * oc:512 * oc + 512],
                                 start=True, stop=False)
                nc.tensor.matmul(ps, xn[1][:, 128 * gpi:128 * gpi + 128],
                                 wq[1][:, 512 * oc:512 * oc + 512],
                                 start=False, stop=True)
                for half in range(2):
                    g = 2 * gpi + half
                    dst = ut[:, 4096 * oc + g: 4096 * oc + g + 4089: 8]
                    src = ps[64 * half:64 * half + 64, :]
                    drain(dst, src, scale=ALPHA)

        # halo rows (j=0): lo rho=7 g=5 ; hi rho=0 g=0
        for e, wcol, gg in ((0, 0, 5), (1, 64, 0)):
            ph = pm.tile([64, 256], FP, tag="m", name=f"phalo{e}")
            nc.tensor.matmul(ph, wch[0][:, wcol:wcol + 64],
                             xn[0][:, 512 + 256 * gg:512 + 256 * gg + 256],
                             start=True, stop=False)
            nc.tensor.matmul(ph, wch[1][:, wcol:wcol + 64],
                             xn[1][:, 512 + 256 * gg:512 + 256 * gg + 256],
                             start=False, stop=True)
            vec.tensor_copy(qhalo[e], ph)

        # ------- conv-input slim projection + per-j permute DMAs --------
        def permute_dmas(srct, dstt, off):
            for ci2 in range(2):
                for hh in range(2):
                    for ya in range(2):
                        src = srct[64 * ya:64 * ya + 64, :].rearrange(
                            "i (r h x) -> h i r x", r=8, h=4, x=64)[2 * ci2 + hh]
                        dst = dstt[ci2][64 * hh:64 * hh + 64,
                                        off + 512 * ya:off + 512 * ya + 512
                                        ].rearrange("i (r x) -> i r x", x=64)
                        dma(out=dst, in_=src)

        for j, dst in ((0, sq2), (1, sk2), (2, sv2)):
            wcgj = [sp.tile([128, 1024], BF, tag="wcgj0", bufs=2,
                            name=f"wcgj0_{j}"),
                    sp.tile([128, 1024], BF, tag="wcgj1", bufs=2,
                            name=f"wcgj1_{j}")]
            for hf in range(2):
                dma(out=wcgj[hf], in_=wcg_d[
                    128 * hf:128 * hf + 128, 1024 * j:1024 * j + 1024])
            for rho in range(8):
                g = (3 * rho + j) % 8
                ps = pm.tile([128, 256], FP, tag="m", name=f"pc{j}_{rho}")
                nc.tensor.matmul(ps, wcgj[0][:, 128 * rho:128 * rho + 128],
                                 xn[0][:, 512 + 256 * g:512 + 256 * g + 256],
                                 start=True, stop=False)
                nc.tensor.matmul(ps, wcgj[1][:, 128 * rho:128 * rho + 128],
                                 xn[1][:, 512 + 256 * g:512 + 256 * g + 256],
                                 start=False, stop=True)
                drain(dst[:, 256 * rho:256 * rho + 256], ps)

        # conv weights (used late; DMA after the slim weights)
        for hf in range(2):
            dma(out=w1s[hf], in_=w1s_d.bitcast(FR)[128 * hf:128 * hf + 128, :])
            dma(out=w2s[hf], in_=w2s_d.bitcast(FR)[128 * hf:128 * hf + 128, :])

        # ---------------- V tiles: pair-sum via fp8 matmul + ones -------
        # vsb[key, e] = 0.25*(V[key, 2e] + V[key, 2e+1]) (alpha folded; the
        # final out stt uses scalar 2.0 to undo 0.25 and apply the 0.5 pool)
        vec.tensor_copy(vsb[:, 32::33], onec.to_broadcast((128, 32)))
        for t in range(32):
            pv = pm.tile([128, 32], FP, tag="m", name=f"vt{t}")
            nc.tensor.matmul(pv, ut[:, 3 * 128 * t + 2: 3 * 128 * t + 384: 3],
                             pairm, start=True, stop=True)
            drain(vsb[:, 33 * t:33 * t + 32], pv)

        # release scratch + front psum; open attention PSUM:
        # 3 double-bank score spools + 1 bank holding three rotating 132-col
        # AV accumulators (explicitly memset, AVs accumulate with
        # start=False) + 1 bank for the conv accumulation
        sctx.close()
        pctx.close()
        spools = [ctx.enter_context(
            tc.tile_pool(name=f"ps_s{k}", bufs=1, space="PSUM"))
            for k in range(3)]
        po = ctx.enter_context(tc.tile_pool(name="ps_o", bufs=1, space="PSUM"))
        pot = po.tile([128, 512], FP, tag="o", name="pav3")
        pc = ctx.enter_context(tc.tile_pool(name="ps_c", bufs=1, space="PSUM"))

        # permute DMAs, emitted after the pool transition so the close-drain
        # does not wait on them; they overlap the attention phase
        for _, srct, dstt, off in ((0, sq2, q2q, 64), (1, sk2, k2q, 0),
                                   (2, sv2, v2q, 0)):
            permute_dmas(srct, dstt, off)
        for ci2 in range(2):
            for hh in range(2):
                for e, dlo, dhi in ((0, 0, 64), (1, 1088, 1152)):
                    src = qhalo[e].rearrange(
                        "i (h x) -> h i x", h=4)[2 * ci2 + hh]
                    dma(out=q2q[ci2][64 * hh:64 * hh + 64, dlo:dhi],
                        in_=src)

        # spatial pre-pooling on Pool (overlaps the attention phase):
        #   q2p[y,X] = q[y,2X]+q[y,2X+1]      (18 rows incl dy halo)
        #   k2E[y,X] = k[y,2X]+k[y,2X+1]
        #   k2O[y,X] = k[y,2X-1]+k[y,2X], X in [0,32] (zero-padded ends)
        #   v2p[e]   = v[2e]+v[2e+1]
        for ci2 in range(2):
            qv = q2q[ci2].rearrange("p (y x two) -> p y x two", two=2, x=32)
            pvq = q2p[ci2].rearrange("p (y x) -> p y x", x=32)
            gp.tensor_tensor(pvq, qv[:, :, :, 0], qv[:, :, :, 1], OP.add)
            kv = k2q[ci2].rearrange("p (y x) -> p y x", x=64)
            ev = k2E[ci2].rearrange("p (y x) -> p y x", x=32)
            kvp = k2q[ci2].rearrange("p (y x two) -> p y x two", two=2, x=32)
            gp.tensor_tensor(ev, kvp[:, :, :, 0], kvp[:, :, :, 1], OP.add)
            ov = k2O[ci2].rearrange("p (y x) -> p y x", x=33)
            gp.tensor_copy(ov[:, :, 0:1], kv[:, :, 0:1])
            gp.tensor_copy(ov[:, :, 32:33], kv[:, :, 63:64])
            gp.tensor_tensor(ov[:, :, 1:32], kv[:, :, 1:62:2],
                             kv[:, :, 2:63:2], OP.add)
            vv = v2q[ci2].rearrange("p (e two) -> p e two", two=2)
            gp.tensor_tensor(v2p[ci2], vv[:, :, 0], vv[:, :, 1], OP.add)

        # ---------------- attention ----------------
        stages = [(c, g) for c in range(8) for g in range(NG)]
        pavs = {c: pot[:, 132 * (c % 3):132 * (c % 3) + 132]
                for c in range(8)}
        for c in range(3):
            act.activation(pavs[c], zconst.to_broadcast((128, 132)), AF.Copy)

        def dr2(ap2, n2):
            # [64, n] strided fp8 slice -> [64, 2, n] with a stride-0 pair
            # dim: DoubleRow contracts both halves, doubling the result.
            return ap2.rearrange("p (one m) -> p one m",
                                 one=1).to_broadcast((64, 2, n2))

        def emit_qk_exp(i):
            c, g = stages[i]
            rhs_q = dr2(ut[:, 3 * 512 * c: 3 * 512 * c + 1534: 3], 512)
            pss = spools[i % 3].tile([128, 1024], FP, tag=f"s{i % 3}",
                                     name=f"s{c}_{g}")
            for q in range(2):
                t = 2 * g + q
                nc.tensor.matmul(
                    pss[:, 512 * q:512 * q + 512],
                    dr2(ut[:, 3 * 128 * t + 1: 3 * 128 * t + 383: 3], 128),
                    rhs_q, start=True, stop=True, skip_group_check=True,
                    perf_mode=PM.DoubleRow)
            psb = psbs[i % (LAG + 1)]
            e = EXPENG[g]
            if e == "A":
                # identical bit-pattern to the DVE fast-exp path, so the
                # approximation bias cancels exactly in the softmax normalizer
                act.activation(psb.bitcast(I16), pss, AF.Copy,
                               bias=EXPB, scale=EXPA)
            else:
                vec.tensor_scalar(psb.bitcast(I16), pss, EXPA, EXPB,
                                  OP.mult, OP.add)

        def emit_av(i):
            c, g = stages[i]
            pav = pavs[c]
            psb = psbs[i % (LAG + 1)]
            for q in range(2):
                t = 2 * g + q
                for s in range(4):
                    # accumulate onto the explicitly-memset region
                    nc.tensor.matmul(
                        pav[:, 33 * s:33 * s + 33],
                        psb[:, 512 * q + 128 * s:512 * q + 128 * s + 128],
                        vsb[:, 33 * t:33 * t + 33],
                        start=False, stop=(t == 31),
                        skip_group_check=True)
        def emit_out_drain(c):
            # per-c output drain, emitted ~8 stages after the last AV so the
            # in-order DVE queue never stalls waiting for the final AV
            pav = pavs[c]
            rec4 = rec4s[c % 2]
            vec.reciprocal(rec4, pav[:, 32::33])
            resb = resbs[c % 2]
            # 2.0 = 0.5 (pool avg) / 0.25 (fp8 drain alpha on V)
            vec.scalar_tensor_tensor(
                out=resb.rearrange("p (s e) -> p s e", s=4),
                in0=pav.rearrange("p (s e) -> p s e", e=33)[:, :, 0:32],
                scalar=2.0, in1=rec4.to_broadcast((128, 4, 32)),
                op0=OP.mult, op1=OP.mult)
            dma(out=out_a[512 * c:512 * c + 512, :].rearrange(
                    "(s p) e -> p s e", s=4),
                in_=resb.rearrange("p (s e) -> p s e", s=4))


        # conv matmuls (pre-pooled), interleaved into the attention stream
        # on a dedicated PSUM bank, two matmuls per stage so PE slack
        # absorbs them without stalling the QK stream.  The [128, 512] bank
        # holds both oc regions of one ch (256 pooled positions = 8 y-rows
        # x 32 X each); single start=True zeroes the bank, oc=1 accumulates
        # onto pending-zero bytes.
        cvs = {}

        def conv_mm_list(ch, oc):
            pvq = [q2p[hf].rearrange("p (y x) -> p y x", x=32)
                   for hf in range(2)]
            ev = [k2E[hf].rearrange("p (y x) -> p y x", x=32)
                  for hf in range(2)]
            ov = [k2O[hf].rearrange("p (y x) -> p y x", x=33)
                  for hf in range(2)]
            ps = pc.tile([128, 256], FP, tag="cv", name=f"cv{ch}{oc}")
            cvs[(ch, oc)] = ps
            mms = []
            k = 0
            for dy in range(3):
                for hf in range(2):
                    mms.append((ps,
                                w1s[hf][:, 256 * dy + 128 * oc:
                                        256 * dy + 128 * oc + 128],
                                pvq[hf][:, 8 * ch + dy:8 * ch + dy + 8, :],
                                k == 0, False))
                    k += 1
            for dx, srcb, xlo in ((0, ov, 0), (1, ev, 0), (2, ov, 1)):
                for hf in range(2):
                    mms.append((ps,
                                w2s[hf][:, 256 * dx + 128 * oc:
                                        256 * dx + 128 * oc + 128],
                                srcb[hf][:, 8 * ch:8 * ch + 8,
                                         xlo:xlo + 32],
                                False, k == 11))
                    k += 1
            return mms

        def emit_conv_drain(ch, oc):
            vec.scalar_tensor_tensor(
                out=pavb[oc][:, 256 * ch:256 * ch + 256],
                in0=cvs[(ch, oc)],
                scalar=0.0,
                in1=v2p[oc][:, 256 * ch:256 * ch + 256],
                op0=OP.add, op1=OP.add)

        CONV0 = 88
        convq = {}
        conv_sched = {}
        conv_drainat = {}
        for u in range(4):
            ch, oc = u // 2, u % 2
            base = CONV0 + u * 8
            for b in range(6):
                conv_sched[base + b] = (ch, oc, b)
            conv_drainat[base + 6] = (ch, oc)
        for i in range(len(stages)):
            emit_qk_exp(i)
            if i >= LAG:
                emit_av(i - LAG)
            if i in conv_sched:
                ch, oc, b = conv_sched[i]
                if b == 0:
                    convq[(ch, oc)] = conv_mm_list(ch, oc)
                for ps, lhsT, rhs, st, sp_ in convq[(ch, oc)][2 * b:2 * b + 2]:
                    nc.tensor.matmul(ps, lhsT, rhs, start=st, stop=sp_,
                                     skip_group_check=True)
            elif i in conv_drainat:
                emit_conv_drain(*conv_drainat[i])
            elif i == CONV0 + 32:
                for oc in range(2):
                    dma(out=out_c.rearrange("(o w) e -> o w e", w=4)[
                            128 * oc:128 * oc + 128, :, :],
                        in_=pavb[oc].rearrange("p (w e) -> p w e", w=4))
            cd = i - 16 - 8  # drain c when reaching stage (c+1, 8)
            if cd >= 0 and cd % 16 == 0:
                emit_out_drain(cd // 16)
            cz = i - 30  # re-zero region (c+3)%3 well after c's drain so
            if cz >= 0 and cz % 16 == 0 and cz // 16 + 3 <= 7:
                # the ACT queue never waits on the DVE-side stt
                act.activation(pavs[cz // 16 + 3],
                               zconst.to_broadcast((128, 132)), AF.Copy)
        for i in range(len(stages) - LAG, len(stages)):
            emit_av(i)
        for c in range(8):
            if 16 * (c + 1) + 8 > len(stages):
                emit_out_drain(c)


# =====================================================================
# Host side
# =====================================================================
_NC_CACHE = None


def _get_nc():
    global _NC_CACHE
    if _NC_CACHE is None:
        _NC_CACHE = build_device_program()
    return _NC_CACHE


def make_in_maps(x, qkv_w, bn_gamma, bn_beta, conv1_w, conv2_w):
    x = np.asarray(x, np.float32)
    WT = np.ascontiguousarray(np.asarray(qkv_w, np.float32).T)   # [256, 1536]
    WTb = WT.astype(ml_dtypes.bfloat16)
    xT = np.ascontiguousarray(x.transpose(0, 2, 1))              # [2, 256, 4096]
    xts = np.ascontiguousarray(
        np.concatenate([xT[0], xT[1]], axis=1).astype(ml_dtypes.float8_e4m3))
    w1s = np.ascontiguousarray(
        0.5 * np.asarray(conv1_w, np.float32)[:, :, :, 0].transpose(1, 2, 0)
        .reshape(256, 768))                                      # [i, dy*256+o]
    w2s = np.ascontiguousarray(
        0.5 * np.asarray(conv2_w, np.float32)[:, :, 0, :].transpose(1, 2, 0)
        .reshape(256, 768))
    gbar = np.ascontiguousarray(
        np.stack([np.asarray(bn_gamma, np.float32),
                  np.asarray(bn_beta, np.float32)], axis=1))     # [256, 2]
    pairm = np.kron(np.eye(32, dtype=np.float32),
                    np.ones((2, 1), np.float32)).astype(
                        ml_dtypes.float8_e4m3)                   # [64, 32]

    ilo = np.arange(64)
    in_maps = []
    for c in range(8):
        b, h = c // 4, c % 4
        head_cols = np.concatenate(
            [512 * g + 64 * h + np.arange(64) for g in range(8)])
        conv_cols = np.concatenate(
            [512 * g + 256 + np.arange(256) for g in range(8)])
        xc = np.ascontiguousarray(
            xT[b][:, np.concatenate([head_cols, conv_cols])]
            .astype(ml_dtypes.bfloat16))                         # [256, 2560]
        # slim conv-proj weights: col (j*8+rho)*128 + 64*mr + ilo
        #   -> WT col (3*rho+j)//8 + 3*(2h+mr) + 24*ilo   (j=2 scaled by 0.5)
        wcg = np.zeros((256, 3072), np.float32)
        for j in range(3):
            sc = 0.5 if j == 2 else 1.0
            for rho in range(8):
                o0 = (3 * rho + j) // 8
                for mr in range(2):
                    cols = o0 + 3 * (2 * h + mr) + 24 * ilo
                    wcg[:, (j * 8 + rho) * 128 + 64 * mr + ilo] = sc * WT[:, cols]
        # halo: lo (rho=7, ya=2h-1): o = 2 + 3*(2h-1) + 24*ilo   (h>=1)
        #       hi (rho=0, ya=2h+2): o = 3*(2h+2) + 24*ilo       (h<=2)
        wch = np.zeros((256, 128), np.float32)
        if h >= 1:
            wch[:, 0:64] = WT[:, 2 + 3 * (2 * h - 1) + 24 * ilo]
        if h <= 2:
            wch[:, 64:128] = WT[:, 3 * (2 * h + 2) + 24 * ilo]
        in_maps.append({
            "xts": xts, "xc": xc, "wq": WTb, "wcg": wcg.astype(ml_dtypes.bfloat16),
            "wch": wch.astype(ml_dtypes.bfloat16),
            "w1s": w1s, "w2s": w2s, "gb": gbar, "pairm": pairm,
        })
    return in_maps


def assemble(results):
    out = np.zeros((B, N, DIM), np.float32)
    for c in range(8):
        b, h = c // 4, c % 4
        out[b, :, 32 * h:32 * h + 32] = results[c]["out_a"]
        oc = results[c]["out_c"].reshape(256, 4, 128)
        out[b].reshape(256, 16, 256)[:, 4 * h:4 * h + 4, 128:256] = oc
    return out


def kernel(**inputs):
    nc = _get_nc()
    in_maps = make_in_maps(**inputs)
    res = bass_utils.run_bass_kernel_spmd(
        nc, in_maps, core_ids=list(range(8)),
        trace=bool(int(os.environ.get("KERNEL_TRACE", "0"))))
    out = assemble(res.results)
    if res.exec_time_ns is not None:
        print(f"HW exec time: {res.exec_time_ns} ns", file=sys.stderr)
        kernel.last_exec_time_ns = res.exec_time_ns
    kernel.last_results = res
    return out


kernel.last_exec_time_ns = None
kernel.last_results = None
